# revision 2
# baseline (speedup 1.0000x reference)
"""Trainium2 Bass kernel for BC_Encoder (MLP + segmented mean/max/min pooling).

Strategy (8-core SPMD, identical program on every core; the program is
JIT-specialized only on the tile count, never on data values):
  - Host packs each core's ~N/8 points into segment-pure 512-point tiles
    (tiles never straddle a segment boundary; short tiles are padded by
    replicating the tile's first point, which is safe for max/min and
    corrected for sums on the host).
  - Device per tile: L1 (K=4: xyz + ones row carrying b1, point-major,
    fp32r matmuls) -> LayerNorm -> ReLU -> L2 (K=256 in two chunks, b2
    added via a K=1 PSUM-init matmul) -> LayerNorm -> ReLU -> L3
    (feature-major).  LN stats via bn_stats/bn_aggr on VectorE, mean/rstd
    folded into the PSUM eviction, fp16 PE-transpose to feature-major.
    Pooling: y3 evicted to fp16 SBUF on ScalarE with a free running sum
    via accum_out; max/min as free-axis reduces on VectorE; the tile's
    first-point column exported for the host-side padding correction.
  - Host un-pads (sum -= n_pad * col0), combines tiles into segments via
    reduceat, divides by true counts, adds b3, concats.

Dispatch: the PJRT executable is traced/compiled ONCE per tile-count and
cached; weights and the packed positions are kept device-resident and
revalidated against the passed-in arrays with cheap equality checks, so
steady-state calls only run the device program, fetch the per-tile
staging columns, and do a vectorized host combine.
"""

import numpy as np

N_CORES = 8
DIN = 3
DINA = 4  # DIN + a constant-ones row carrying b1
H = 256
EPS = 1e-5
TILE = 512
PB = 128
NPB = TILE // PB  # point-blocks per tile

_PROGRAMS = {}  # nt -> compiled Bass module
_RUNNERS = {}  # nt -> dict(sharded, zeros_fn, in_names, shard)
_STATE = {}  # single-slot input-derived caches


def _build_program(nt):
    import concourse.bass as bass  # noqa: F401 (registers ops)
    import concourse.tile as tile
    from concourse import bacc, mybir
    from concourse.masks import make_identity

    f32 = mybir.dt.float32
    f16 = mybir.dt.float16
    f32r = mybir.dt.float32r

    nc = bacc.Bacc("TRN2", target_bir_lowering=False, debug=False)

    posT = nc.dram_tensor("posT", [DINA, nt * TILE], f32r, kind="ExternalInput")
    w1t = nc.dram_tensor("w1t", [DINA, H], f32r, kind="ExternalInput")
    w2t = nc.dram_tensor("w2t", [H, H], f32r, kind="ExternalInput")
    w3t = nc.dram_tensor("w3t", [H, H], f32r, kind="ExternalInput")
    b2r = nc.dram_tensor("b2r", [1, H], f32r, kind="ExternalInput")
    onesr = nc.dram_tensor("onesr", [1, PB], f32r, kind="ExternalInput")
    gbe = nc.dram_tensor("gbe", [H, 4], f32, kind="ExternalInput")
    stag_d = nc.dram_tensor("stag", [8, PB, nt], f32, kind="ExternalOutput")

    def r(ap):
        return ap if ap.dtype == f32r else ap.bitcast(f32r)

    with tile.TileContext(nc) as tc:
        with (
            tc.tile_pool(name="consts", bufs=1) as consts,
            tc.tile_pool(name="xin", bufs=4) as xin,
            tc.tile_pool(name="tsb", bufs=2) as tsb,
            tc.tile_pool(name="zsb", bufs=3) as zsb,
            tc.tile_pool(name="stats", bufs=4) as stats_p,
            tc.tile_pool(name="psy", bufs=2, space="PSUM") as psy,
            tc.tile_pool(name="pstt", bufs=2, space="PSUM") as pstt,
            tc.tile_pool(name="psy3", bufs=1, space="PSUM") as psy3,
        ):
            # ---- constants ----
            w1_sb = consts.tile([DINA, H], f32r)
            nc.sync.dma_start(w1_sb[:], w1t[:])
            b2_sb = consts.tile([1, H], f32r)
            nc.sync.dma_start(b2_sb[:], b2r[:])
            ones1 = consts.tile([1, PB], f32r)
            nc.sync.dma_start(ones1[:], onesr[:])
            w2_sb = [consts.tile([PB, H], f32r, tag=f"w2_{k}", name=f"w2_{k}") for k in range(2)]
            for k in range(2):
                nc.sync.dma_start(w2_sb[k][:], w2t[k * PB : (k + 1) * PB, :])
            w3_sb = [
                [consts.tile([PB, PB], f32r, tag=f"w3_{k}{m}", name=f"w3_{k}{m}") for m in range(2)]
                for k in range(2)
            ]
            for k in range(2):
                for m in range(2):
                    nc.sync.dma_start(
                        w3_sb[k][m][:],
                        w3t[k * PB : (k + 1) * PB, m * PB : (m + 1) * PB],
                    )
            gbe_sb = [consts.tile([PB, 4], f32, tag=f"gbe_{fb}", name=f"gbe_{fb}") for fb in range(2)]
            for fb in range(2):
                nc.sync.dma_start(gbe_sb[fb][:], gbe[fb * PB : (fb + 1) * PB, :])
            eps_sb = consts.tile([PB, 1], f32)
            nc.vector.memset(eps_sb[:], EPS)
            ident = consts.tile([PB, PB], f16)
            make_identity(nc, ident[:])
            # staging accumulators (written column-by-column, DMA'd at end)
            stag = [consts.tile([PB, nt], f32, tag=f"stag_{i}", name=f"stag_{i}") for i in range(8)]

            def layer_norm(y_ps, gbe_cols, z_out):
                """y_ps: PSUM [PB, NPB, H] point-major. Writes z_out [PB, 2, TILE]
                feature-major = relu(LN(y) * g + be)."""
                st = stats_p.tile([PB, NPB, 6], f32, tag="bn6")
                for pb in range(NPB):
                    nc.vector.bn_stats(st[:, pb, :], y_ps[:, pb, :])
                mv = stats_p.tile([PB, NPB, 2], f32, tag="mv")
                for pb in range(NPB):
                    nc.vector.bn_aggr(mv[:, pb, :], st[:, pb, :])
                rstd = stats_p.tile([PB, NPB], f32, tag="rstd")
                nc.scalar.activation(
                    rstd[:], mv[:, :, 1], mybir.ActivationFunctionType.Sqrt,
                    bias=eps_sb[:], scale=1.0,
                )
                nc.vector.reciprocal(rstd[:], rstd[:])
                nmr = stats_p.tile([PB, NPB], f32, tag="nmr")
                nc.vector.tensor_mul(nmr[:], mv[:, :, 0], rstd[:])
                nc.vector.tensor_scalar_mul(nmr[:], nmr[:], -1.0)
                # evict with per-point (partition) normalization, fp16 out;
                # split across ScalarE (scale/bias form) and VectorE (2-op form)
                t_sb = tsb.tile([PB, NPB, H], f16, tag="t")
                for pb in range(NPB):
                    if pb % 2 == 0:
                        nc.scalar.activation(
                            t_sb[:, pb, :], y_ps[:, pb, :],
                            mybir.ActivationFunctionType.Identity,
                            bias=nmr[:, pb : pb + 1], scale=rstd[:, pb : pb + 1],
                        )
                    else:
                        nc.vector.tensor_scalar(
                            t_sb[:, pb, :], y_ps[:, pb, :],
                            mv[:, pb, 0:1], rstd[:, pb : pb + 1],
                            mybir.AluOpType.subtract, mybir.AluOpType.mult,
                        )
                # transpose to feature-major, then gamma/beta/relu application
                for fb in range(2):
                    tt = pstt.tile([PB, TILE], f16, tag="tt")
                    for pb in range(NPB):
                        nc.tensor.transpose(
                            tt[:, pb * PB : (pb + 1) * PB],
                            t_sb[:, pb, fb * PB : (fb + 1) * PB],
                            ident[:],
                        )
                    nc.scalar.activation(
                        z_out[:, fb, :], tt[:],
                        mybir.ActivationFunctionType.Relu,
                        bias=gbe_cols[fb][1], scale=gbe_cols[fb][0],
                    )

            for t in range(nt):
                x0 = xin.tile([DINA, TILE], f32r, tag="x0")
                nc.sync.dma_start(x0[:], posT[:, t * TILE : (t + 1) * TILE])

                # ---- L1 (point-major, K=4: xyz + ones row carrying b1) ----
                y1 = psy.tile([PB, NPB, H], f32, tag="y")
                for pb in range(NPB):
                    nc.tensor.matmul(
                        y1[:, pb, :], r(x0[:, pb * PB : (pb + 1) * PB]), r(w1_sb[:]),
                        start=True, stop=True,
                    )
                z1 = zsb.tile([PB, 2, TILE], f32r, tag="z")
                layer_norm(
                    y1,
                    [(gbe_sb[fb][:, 0:1], gbe_sb[fb][:, 1:2]) for fb in range(2)],
                    z1,
                )

                # ---- L2 (point-major, K=256 in two chunks; b2 via K=1 init) ----
                y2 = psy.tile([PB, NPB, H], f32, tag="y")
                for pb in range(NPB):
                    nc.tensor.matmul(
                        y2[:, pb, :], r(ones1[:]), r(b2_sb[:]),
                        start=True, stop=False,
                    )
                    for k in range(2):
                        nc.tensor.matmul(
                            y2[:, pb, :],
                            r(z1[:, k, pb * PB : (pb + 1) * PB]),
                            r(w2_sb[k][:]),
                            start=False, stop=(k == 1),
                        )
                z2 = zsb.tile([PB, 2, TILE], f32r, tag="z")
                layer_norm(
                    y2,
                    [(gbe_sb[fb][:, 2:3], gbe_sb[fb][:, 3:4]) for fb in range(2)],
                    z2,
                )

                # ---- L3 (feature-major: out [h-block, pts]) ----
                y3 = [psy3.tile([PB, TILE], f32, tag=f"y3_{m}", name=f"y3_{m}") for m in range(2)]
                for m in range(2):
                    for k in range(2):
                        nc.tensor.matmul(
                            y3[m][:], r(w3_sb[k][m][:]), r(z2[:, k, :]),
                            start=(k == 0), stop=(k == 1),
                        )

                # ---- per-tile pooling columns ----
                X = mybir.AxisListType.X
                z3 = zsb.tile([PB, 2, TILE], f16, tag="z3")
                for m in range(2):
                    nc.scalar.activation(
                        z3[:, m, :], y3[m][:],
                        mybir.ActivationFunctionType.Identity,
                        bias=0.0, scale=1.0,
                        accum_out=stag[0 + m][:, t : t + 1],
                    )
                    nc.vector.tensor_reduce(
                        stag[2 + m][:, t : t + 1], z3[:, m, :], axis=X,
                        op=mybir.AluOpType.max,
                    )
                    nc.vector.tensor_reduce(
                        stag[4 + m][:, t : t + 1], z3[:, m, :], axis=X,
                        op=mybir.AluOpType.min,
                    )
                    nc.gpsimd.tensor_copy(stag[6 + m][:, t : t + 1], z3[:, m, 0:1])

            for i in range(8):
                nc.sync.dma_start(stag_d[i], stag[i][:])

    nc.compile()
    return nc


def _make_runner(nc):
    """Build a cached PJRT executable for the Bass module: jit the shard_map
    body ONCE so later calls skip retracing/relowering (the stock
    run_bass_kernel_spmd path re-jits on every invocation)."""
    import jax
    import jax.numpy as jnp
    from jax.experimental.shard_map import shard_map
    from jax.sharding import Mesh, NamedSharding, PartitionSpec

    from concourse import mybir
    from concourse.bass2jax import (
        _bass_exec_p,
        install_neuronx_cc_hook,
        partition_id_tensor,
    )

    install_neuronx_cc_hook()
    partition_name = nc.partition_id_tensor.name if nc.partition_id_tensor else None

    in_names, out_names, out_avals, zero_shapes = [], [], [], []
    for alloc in nc.m.functions[0].allocations:
        if not isinstance(alloc, mybir.MemoryLocationSet):
            continue
        name = alloc.memorylocations[0].name
        if alloc.kind == "ExternalInput":
            if name != partition_name:
                in_names.append(name)
        elif alloc.kind == "ExternalOutput":
            shape = tuple(alloc.tensor_shape)
            dtype = mybir.dt.np(alloc.dtype)
            out_names.append(name)
            out_avals.append(jax.core.ShapedArray(shape, dtype))
            zero_shapes.append((shape, dtype))
    n_params = len(in_names)
    n_outs = len(out_names)
    all_names = list(in_names) + list(out_names)
    if partition_name is not None:
        all_names.append(partition_name)
    donate = tuple(range(n_params, n_params + n_outs))

    def _body(*args):
        operands = list(args)
        if partition_name is not None:
            operands.append(partition_id_tensor())
        outs = _bass_exec_p.bind(
            *operands,
            out_avals=tuple(out_avals),
            in_names=tuple(all_names),
            out_names=tuple(out_names),
            lowering_input_output_aliases=(),
            sim_require_finite=True,
            sim_require_nnan=True,
            nc=nc,
        )
        return tuple(outs)

    devices = jax.devices()[:N_CORES]
    mesh = Mesh(np.asarray(devices), ("core",))
    spec = PartitionSpec("core")
    sharded = jax.jit(
        shard_map(
            _body,
            mesh=mesh,
            in_specs=(spec,) * (n_params + n_outs),
            out_specs=(spec,) * n_outs,
            check_rep=False,
        ),
        donate_argnums=donate,
        keep_unused=True,
    )
    shard = NamedSharding(mesh, spec)
    gshapes = [((N_CORES * s[0],) + tuple(s[1:]), d) for s, d in zero_shapes]
    zeros_fn = jax.jit(
        lambda: tuple(jnp.zeros(s, d) for s, d in gshapes),
        out_shardings=(shard,) * n_outs,
    )
    return dict(
        sharded=sharded,
        zeros_fn=zeros_fn,
        in_names=in_names,
        out_names=out_names,
        shard=shard,
    )


def _host_prep(bi):
    """Pack points into segment-pure tiles per core; precompute the
    tile->segment combine metadata. Pure function of batch_index."""
    n = bi.shape[0]
    edges = [c * n // N_CORES for c in range(N_CORES + 1)]
    cores = []
    for c in range(N_CORES):
        lo, hi = edges[c], edges[c + 1]
        segs = bi[lo:hi]
        cuts = np.flatnonzero(np.diff(segs)) + 1 + lo
        bounds = np.concatenate([[lo], cuts, [hi]])
        idx_parts, tmap, n_real = [], [], []
        for j in range(len(bounds) - 1):
            s, e = int(bounds[j]), int(bounds[j + 1])
            seg = int(bi[s])
            for ts in range(s, e, TILE):
                te = min(ts + TILE, e)
                k = te - ts
                part = np.arange(ts, te, dtype=np.int64)
                if k < TILE:
                    part = np.concatenate([part, np.full(TILE - k, ts, np.int64)])
                idx_parts.append(part)
                tmap.append(seg)
                n_real.append(k)
        cores.append((idx_parts, tmap, n_real))
    nt = max(len(cc[1]) for cc in cores)
    idx_all = np.zeros((N_CORES, nt * TILE), np.int64)
    tmap_all = np.full((N_CORES, nt), -1, np.int64)
    nreal_all = np.zeros((N_CORES, nt), np.int64)
    for c, (idx_parts, tmap, n_real) in enumerate(cores):
        k = len(tmap)
        if k:
            idx_all[c, : k * TILE] = np.concatenate(idx_parts)
            tmap_all[c, :k] = tmap
            nreal_all[c, :k] = n_real
    tmap_flat = tmap_all.ravel()
    live = np.flatnonzero(tmap_flat >= 0)
    order = np.argsort(tmap_flat[live], kind="stable")
    ordered = live[order]
    sseg = tmap_flat[ordered]
    starts = np.flatnonzero(np.r_[True, np.diff(sseg) > 0])
    counts = np.bincount(bi.astype(np.int64))
    return dict(
        nt=nt,
        idx_flat=idx_all.ravel(),
        cores_ord=(ordered // nt).astype(np.intp),
        tiles_ord=(ordered % nt).astype(np.intp),
        starts=starts,
        seg_ids=sseg[starts].astype(np.intp),
        npad_ord=(TILE - nreal_all.ravel()[ordered]).astype(np.float32),
        counts=counts,
    )


def kernel(
    positions, W1, b1, W2, b2, W3, b3, g1, be1, g2, be2, batch_index, num_segments
):
    import jax

    positions = np.asarray(positions, np.float32)
    weights = tuple(
        np.asarray(a, np.float32) for a in (W1, b1, W2, b2, W3, b3, g1, be1, g2, be2)
    )
    W1, b1, W2, b2, W3, b3, g1, be1, g2, be2 = weights
    bi = np.asarray(batch_index)
    B = int(num_segments)
    st = _STATE

    # ---- packing plan (cached on batch_index equality) ----
    if (
        st.get("bi") is None
        or bi.shape != st["bi"].shape
        or not np.array_equal(bi, st["bi"])
    ):
        st["bi"] = bi.copy()
        st["prep"] = _host_prep(st["bi"])
        st.pop("pos_copy", None)  # packed positions depend on the plan
    prep = st["prep"]
    nt = prep["nt"]

    if nt not in _PROGRAMS:
        _PROGRAMS[nt] = _build_program(nt)
        _RUNNERS[nt] = _make_runner(_PROGRAMS[nt])
    run = _RUNNERS[nt]

    # ---- weights: device-resident, revalidated by equality ----
    w_sig = st.get("w_sig")
    if (
        w_sig is None
        or w_sig[0] != nt
        or not all(np.array_equal(a, c) for a, c in zip(weights, w_sig[1]))
    ):
        st["w_sig"] = (nt, tuple(a.copy() for a in weights))
        reps = {
            "w1t": np.ascontiguousarray(np.concatenate([W1.T, b1[None, :]], axis=0)),
            "w2t": np.ascontiguousarray(W2.T),
            "w3t": np.ascontiguousarray(W3.T),
            "b2r": np.ascontiguousarray(b2[None, :]),
            "onesr": np.ones((1, PB), np.float32),
            "gbe": np.ascontiguousarray(np.stack([g1, be1, g2, be2], axis=1)),
        }
        st["w_dev"] = {
            k: jax.device_put(
                np.concatenate([v] * N_CORES, axis=0), run["shard"]
            )
            for k, v in reps.items()
        }

    # ---- packed positions: device-resident, revalidated by equality ----
    if (
        st.get("pos_copy") is None
        or positions.shape != st["pos_copy"].shape
        or not np.array_equal(positions, st["pos_copy"])
    ):
        st["pos_copy"] = positions.copy()
        L = nt * TILE
        posT = np.empty((N_CORES, DINA, L), np.float32)
        posT[:, DIN, :] = 1.0
        g = positions[prep["idx_flat"]]
        posT[:, :DIN, :] = g.reshape(N_CORES, L, DIN).transpose(0, 2, 1)
        st["posT_dev"] = jax.device_put(posT.reshape(N_CORES * DINA, L), run["shard"])

    # ---- execute ----
    ins = {"posT": st["posT_dev"], **st["w_dev"]}
    args = [ins[n] for n in run["in_names"]]
    zeros = run["zeros_fn"]()
    outs = run["sharded"](*args, *zeros)
    stag = np.asarray(outs[0])  # [N_CORES*8, PB, nt]

    # ---- vectorized unshard / segment combine ----
    A = stag.reshape(N_CORES, 8, PB, nt)
    co, to = prep["cores_ord"], prep["tiles_ord"]
    s_sel = A[:, 0:2].reshape(N_CORES, H, nt)[co, :, to]
    mx_sel = A[:, 2:4].reshape(N_CORES, H, nt)[co, :, to]
    mn_sel = A[:, 4:6].reshape(N_CORES, H, nt)[co, :, to]
    c0_sel = A[:, 6:8].reshape(N_CORES, H, nt)[co, :, to]
    s_corr = (s_sel - prep["npad_ord"][:, None] * c0_sel).astype(np.float64)
    starts = prep["starts"]
    sums = np.add.reduceat(s_corr, starts, axis=0)
    maxs = np.maximum.reduceat(mx_sel, starts, axis=0)
    mins = np.minimum.reduceat(mn_sel, starts, axis=0)

    counts = prep["counts"]
    if counts.shape[0] < B:
        counts = np.pad(counts, (0, B - counts.shape[0]))
    seg_ids = prep["seg_ids"]
    out = np.zeros((B, 3 * H), np.float32)
    cnt = np.maximum(counts[seg_ids], 1).astype(np.float64)
    out[seg_ids, 0:H] = (sums / cnt[:, None]).astype(np.float32)
    out[seg_ids, H : 2 * H] = maxs
    out[seg_ids, 2 * H : 3 * H] = mins
    out += np.tile(b3, 3)[None, :]
    return out


# revision 3
# speedup vs baseline: 3.3747x; 3.3747x over previous
"""Trainium2 Bass kernel for BC_Encoder (MLP + segmented mean/max/min pooling).

Strategy (8-core SPMD, identical program on every core; the program is
JIT-specialized on (tiles-per-segment, segments-per-core), never on data
values):
  - Each segment is assigned to exactly ONE core (segments-per-core =
    ceil(B/8); segment sizes are near-uniform so cores stay balanced).
    Each segment is padded to a FIXED number of 512-point tiles (TPS =
    ceil(max_segment/512)), so the tile->segment map is static. Pad
    slots replicate the owning tile's first point (a real point of the
    segment), which is safe for max/min; for sums the device exports
    n_pad * y(first point) per tile (n_pad arrives as an input tensor)
    and subtracts it, making per-segment sums exact.
  - Device per tile: L1 (K=4: xyz + ones row carrying b1, point-major,
    fp32r matmuls) -> LayerNorm -> ReLU -> L2 (K=256 in two chunks, b2
    added via a K=1 PSUM-init matmul) -> LayerNorm -> ReLU -> L3
    (feature-major).  LN stats via bn_stats/bn_aggr on VectorE, mean/rstd
    folded into the PSUM eviction, fp16 PE-transpose to feature-major.
    Pooling: y3 evicted to fp16 SBUF on ScalarE with a free running sum
    via accum_out; max/min as free-axis reduces on VectorE.
  - Device epilogue: per-segment sum/max/min over the segment's STATIC
    tile-column range, so the output is only [6, 128, SPC] per core
    (~24KB) instead of per-tile columns (~1MB) — the device->host fetch
    through the axon tunnel (~18ms/MB + ~80ms fixed) dominates wall
    time, so output bytes matter far more than device cycles.
  - Host divides by true counts, adds b3, concats. All per-call host
    work is vectorized; the packing plan, device-resident weights and
    packed positions are cached and revalidated with cheap equality
    checks, and the PJRT executable is traced/compiled once and reused
    (the stock run_bass_kernel_spmd path re-jits every invocation).
"""

import numpy as np

N_CORES = 8
DIN = 3
DINA = 4  # DIN + a constant-ones row carrying b1
H = 256
EPS = 1e-5
TILE = 512
PB = 128
NPB = TILE // PB  # point-blocks per tile

_PROGRAMS = {}  # (TPS, SPC) -> compiled Bass module
_RUNNERS = {}  # (TPS, SPC) -> dict(sharded, zeros_fn, in_names, shard)
_STATE = {}  # single-slot input-derived caches


def _build_program(TPS, SPC):
    import concourse.bass as bass  # noqa: F401 (registers ops)
    import concourse.tile as tile
    from concourse import bacc, mybir
    from concourse.masks import make_identity

    f32 = mybir.dt.float32
    f16 = mybir.dt.float16
    f32r = mybir.dt.float32r
    nt = TPS * SPC

    nc = bacc.Bacc("TRN2", target_bir_lowering=False, debug=False)

    posT = nc.dram_tensor("posT", [DINA, nt * TILE], f32r, kind="ExternalInput")
    w1t = nc.dram_tensor("w1t", [DINA, H], f32r, kind="ExternalInput")
    w2t = nc.dram_tensor("w2t", [H, H], f32r, kind="ExternalInput")
    w3t = nc.dram_tensor("w3t", [H, H], f32r, kind="ExternalInput")
    b2r = nc.dram_tensor("b2r", [1, H], f32r, kind="ExternalInput")
    onesr = nc.dram_tensor("onesr", [1, PB], f32r, kind="ExternalInput")
    gbe = nc.dram_tensor("gbe", [H, 4], f32, kind="ExternalInput")
    npadb = nc.dram_tensor("npadb", [PB, nt], f16, kind="ExternalInput")
    out_d = nc.dram_tensor("outp", [6, PB, SPC], f32, kind="ExternalOutput")

    def r(ap):
        return ap if ap.dtype == f32r else ap.bitcast(f32r)

    with tile.TileContext(nc) as tc:
        with (
            tc.tile_pool(name="consts", bufs=1) as consts,
            tc.tile_pool(name="xin", bufs=4) as xin,
            tc.tile_pool(name="tsb", bufs=2) as tsb,
            tc.tile_pool(name="zsb", bufs=3) as zsb,
            tc.tile_pool(name="stats", bufs=4) as stats_p,
            tc.tile_pool(name="epi", bufs=1) as epi,
            tc.tile_pool(name="psy", bufs=2, space="PSUM") as psy,
            tc.tile_pool(name="pstt", bufs=2, space="PSUM") as pstt,
            tc.tile_pool(name="psy3", bufs=1, space="PSUM") as psy3,
        ):
            # ---- constants ----
            w1_sb = consts.tile([DINA, H], f32r)
            nc.sync.dma_start(w1_sb[:], w1t[:])
            b2_sb = consts.tile([1, H], f32r)
            nc.sync.dma_start(b2_sb[:], b2r[:])
            ones1 = consts.tile([1, PB], f32r)
            nc.sync.dma_start(ones1[:], onesr[:])
            w2_sb = [consts.tile([PB, H], f32r, tag=f"w2_{k}", name=f"w2_{k}") for k in range(2)]
            for k in range(2):
                nc.sync.dma_start(w2_sb[k][:], w2t[k * PB : (k + 1) * PB, :])
            w3_sb = [
                [consts.tile([PB, PB], f32r, tag=f"w3_{k}{m}", name=f"w3_{k}{m}") for m in range(2)]
                for k in range(2)
            ]
            for k in range(2):
                for m in range(2):
                    nc.sync.dma_start(
                        w3_sb[k][m][:],
                        w3t[k * PB : (k + 1) * PB, m * PB : (m + 1) * PB],
                    )
            gbe_sb = [consts.tile([PB, 4], f32, tag=f"gbe_{fb}", name=f"gbe_{fb}") for fb in range(2)]
            for fb in range(2):
                nc.sync.dma_start(gbe_sb[fb][:], gbe[fb * PB : (fb + 1) * PB, :])
            eps_sb = consts.tile([PB, 1], f32)
            nc.vector.memset(eps_sb[:], EPS)
            ident = consts.tile([PB, PB], f16)
            make_identity(nc, ident[:])
            npad_sb = consts.tile([PB, nt], f16, tag="npad", name="npad")
            nc.sync.dma_start(npad_sb[:], npadb[:])
            # staging accumulators (written column-by-column, reduced at end)
            stag = [consts.tile([PB, nt], f32, tag=f"stag_{i}", name=f"stag_{i}") for i in range(8)]

            def layer_norm(y_ps, gbe_cols, z_out):
                """y_ps: PSUM [PB, NPB, H] point-major. Writes z_out [PB, 2, TILE]
                feature-major = relu(LN(y) * g + be)."""
                st = stats_p.tile([PB, NPB, 6], f32, tag="bn6")
                for pb in range(NPB):
                    nc.vector.bn_stats(st[:, pb, :], y_ps[:, pb, :])
                mv = stats_p.tile([PB, NPB, 2], f32, tag="mv")
                for pb in range(NPB):
                    nc.vector.bn_aggr(mv[:, pb, :], st[:, pb, :])
                rstd = stats_p.tile([PB, NPB], f32, tag="rstd")
                nc.scalar.activation(
                    rstd[:], mv[:, :, 1], mybir.ActivationFunctionType.Sqrt,
                    bias=eps_sb[:], scale=1.0,
                )
                nc.vector.reciprocal(rstd[:], rstd[:])
                nmr = stats_p.tile([PB, NPB], f32, tag="nmr")
                nc.vector.tensor_mul(nmr[:], mv[:, :, 0], rstd[:])
                nc.vector.tensor_scalar_mul(nmr[:], nmr[:], -1.0)
                # evict with per-point (partition) normalization, fp16 out;
                # split across ScalarE (scale/bias form) and VectorE (2-op form)
                t_sb = tsb.tile([PB, NPB, H], f16, tag="t")
                for pb in range(NPB):
                    if pb % 2 == 0:
                        nc.scalar.activation(
                            t_sb[:, pb, :], y_ps[:, pb, :],
                            mybir.ActivationFunctionType.Identity,
                            bias=nmr[:, pb : pb + 1], scale=rstd[:, pb : pb + 1],
                        )
                    else:
                        nc.vector.tensor_scalar(
                            t_sb[:, pb, :], y_ps[:, pb, :],
                            mv[:, pb, 0:1], rstd[:, pb : pb + 1],
                            mybir.AluOpType.subtract, mybir.AluOpType.mult,
                        )
                # transpose to feature-major, then gamma/beta/relu application
                for fb in range(2):
                    tt = pstt.tile([PB, TILE], f16, tag="tt")
                    for pb in range(NPB):
                        nc.tensor.transpose(
                            tt[:, pb * PB : (pb + 1) * PB],
                            t_sb[:, pb, fb * PB : (fb + 1) * PB],
                            ident[:],
                        )
                    nc.scalar.activation(
                        z_out[:, fb, :], tt[:],
                        mybir.ActivationFunctionType.Relu,
                        bias=gbe_cols[fb][1], scale=gbe_cols[fb][0],
                    )

            for t in range(nt):
                x0 = xin.tile([DINA, TILE], f32r, tag="x0")
                nc.sync.dma_start(x0[:], posT[:, t * TILE : (t + 1) * TILE])

                # ---- L1 (point-major, K=4: xyz + ones row carrying b1) ----
                y1 = psy.tile([PB, NPB, H], f32, tag="y")
                for pb in range(NPB):
                    nc.tensor.matmul(
                        y1[:, pb, :], r(x0[:, pb * PB : (pb + 1) * PB]), r(w1_sb[:]),
                        start=True, stop=True,
                    )
                z1 = zsb.tile([PB, 2, TILE], f32r, tag="z")
                layer_norm(
                    y1,
                    [(gbe_sb[fb][:, 0:1], gbe_sb[fb][:, 1:2]) for fb in range(2)],
                    z1,
                )

                # ---- L2 (point-major, K=256 in two chunks; b2 via K=1 init) ----
                y2 = psy.tile([PB, NPB, H], f32, tag="y")
                for pb in range(NPB):
                    nc.tensor.matmul(
                        y2[:, pb, :], r(ones1[:]), r(b2_sb[:]),
                        start=True, stop=False,
                    )
                    for k in range(2):
                        nc.tensor.matmul(
                            y2[:, pb, :],
                            r(z1[:, k, pb * PB : (pb + 1) * PB]),
                            r(w2_sb[k][:]),
                            start=False, stop=(k == 1),
                        )
                z2 = zsb.tile([PB, 2, TILE], f32r, tag="z")
                layer_norm(
                    y2,
                    [(gbe_sb[fb][:, 2:3], gbe_sb[fb][:, 3:4]) for fb in range(2)],
                    z2,
                )

                # ---- L3 (feature-major: out [h-block, pts]) ----
                y3 = [psy3.tile([PB, TILE], f32, tag=f"y3_{m}", name=f"y3_{m}") for m in range(2)]
                for m in range(2):
                    for k in range(2):
                        nc.tensor.matmul(
                            y3[m][:], r(w3_sb[k][m][:]), r(z2[:, k, :]),
                            start=(k == 0), stop=(k == 1),
                        )

                # ---- per-tile pooling columns ----
                X = mybir.AxisListType.X
                z3 = zsb.tile([PB, 2, TILE], f16, tag="z3")
                for m in range(2):
                    nc.scalar.activation(
                        z3[:, m, :], y3[m][:],
                        mybir.ActivationFunctionType.Identity,
                        bias=0.0, scale=1.0,
                        accum_out=stag[0 + m][:, t : t + 1],
                    )
                    nc.vector.tensor_reduce(
                        stag[2 + m][:, t : t + 1], z3[:, m, :], axis=X,
                        op=mybir.AluOpType.max,
                    )
                    nc.vector.tensor_reduce(
                        stag[4 + m][:, t : t + 1], z3[:, m, :], axis=X,
                        op=mybir.AluOpType.min,
                    )
                    # n_pad * y(first point of tile): the host subtracts this
                    # from the tile sum (pad slots replicate the first point)
                    nc.gpsimd.tensor_mul(
                        stag[6 + m][:, t : t + 1], z3[:, m, 0:1],
                        npad_sb[:, t : t + 1],
                    )

            # ---- epilogue: per-segment combine over static tile ranges ----
            X = mybir.AxisListType.X
            sc = epi.tile([PB, 2, nt], f32, tag="sc")
            for m in range(2):
                nc.vector.tensor_sub(sc[:, m, :], stag[0 + m][:], stag[6 + m][:])
            res = epi.tile([PB, 6, SPC], f32, tag="res")
            for m in range(2):
                for s in range(SPC):
                    sl = slice(s * TPS, (s + 1) * TPS)
                    nc.vector.tensor_reduce(
                        res[:, 0 + m, s : s + 1], sc[:, m, sl], axis=X,
                        op=mybir.AluOpType.add,
                    )
                    nc.vector.tensor_reduce(
                        res[:, 2 + m, s : s + 1], stag[2 + m][:, sl], axis=X,
                        op=mybir.AluOpType.max,
                    )
                    nc.vector.tensor_reduce(
                        res[:, 4 + m, s : s + 1], stag[4 + m][:, sl], axis=X,
                        op=mybir.AluOpType.min,
                    )
            for j in range(6):
                nc.sync.dma_start(out_d[j], res[:, j, :])

    nc.compile()
    return nc


def _make_runner(nc):
    """Build a cached PJRT executable for the Bass module: jit the shard_map
    body ONCE so later calls skip retracing/relowering (the stock
    run_bass_kernel_spmd path re-jits on every invocation)."""
    import jax
    import jax.numpy as jnp
    from jax.experimental.shard_map import shard_map
    from jax.sharding import Mesh, NamedSharding, PartitionSpec

    from concourse import mybir
    from concourse.bass2jax import (
        _bass_exec_p,
        install_neuronx_cc_hook,
        partition_id_tensor,
    )

    install_neuronx_cc_hook()
    partition_name = nc.partition_id_tensor.name if nc.partition_id_tensor else None

    in_names, out_names, out_avals, zero_shapes = [], [], [], []
    for alloc in nc.m.functions[0].allocations:
        if not isinstance(alloc, mybir.MemoryLocationSet):
            continue
        name = alloc.memorylocations[0].name
        if alloc.kind == "ExternalInput":
            if name != partition_name:
                in_names.append(name)
        elif alloc.kind == "ExternalOutput":
            shape = tuple(alloc.tensor_shape)
            dtype = mybir.dt.np(alloc.dtype)
            out_names.append(name)
            out_avals.append(jax.core.ShapedArray(shape, dtype))
            zero_shapes.append((shape, dtype))
    n_params = len(in_names)
    n_outs = len(out_names)
    all_names = list(in_names) + list(out_names)
    if partition_name is not None:
        all_names.append(partition_name)
    donate = tuple(range(n_params, n_params + n_outs))

    def _body(*args):
        operands = list(args)
        if partition_name is not None:
            operands.append(partition_id_tensor())
        outs = _bass_exec_p.bind(
            *operands,
            out_avals=tuple(out_avals),
            in_names=tuple(all_names),
            out_names=tuple(out_names),
            lowering_input_output_aliases=(),
            sim_require_finite=True,
            sim_require_nnan=True,
            nc=nc,
        )
        return tuple(outs)

    devices = jax.devices()[:N_CORES]
    mesh = Mesh(np.asarray(devices), ("core",))
    spec = PartitionSpec("core")
    sharded = jax.jit(
        shard_map(
            _body,
            mesh=mesh,
            in_specs=(spec,) * (n_params + n_outs),
            out_specs=(spec,) * n_outs,
            check_rep=False,
        ),
        donate_argnums=donate,
        keep_unused=True,
    )
    shard = NamedSharding(mesh, spec)
    gshapes = [((N_CORES * s[0],) + tuple(s[1:]), d) for s, d in zero_shapes]
    zeros_fn = jax.jit(
        lambda: tuple(jnp.zeros(s, d) for s, d in gshapes),
        out_shardings=(shard,) * n_outs,
    )
    return dict(
        sharded=sharded,
        zeros_fn=zeros_fn,
        in_names=in_names,
        out_names=out_names,
        shard=shard,
    )


def _host_prep(bi, B):
    """Segment-per-core packing plan. Pure function of (batch_index, B).

    Each core owns SPC consecutive segments; each segment owns TPS
    consecutive tile slots. Pad slots replicate the owning tile's first
    point. Returns gather indices, the per-tile pad-count tensor, and
    per-segment counts."""
    counts = np.bincount(bi.astype(np.int64), minlength=B)
    assert counts.shape[0] == B, "batch_index values must lie in [0, num_segments)"
    SPC = -(-B // N_CORES)
    TPS = max(1, -(-int(counts.max()) // TILE))
    nt = SPC * TPS
    offs = np.concatenate([[0], np.cumsum(counts)])
    idx = np.zeros((N_CORES, nt, TILE), np.int64)
    npad = np.zeros((N_CORES, nt), np.float16)
    for c in range(N_CORES):
        for sl in range(SPC):
            seg = c * SPC + sl
            if seg >= B:
                npad[c, sl * TPS : (sl + 1) * TPS] = 0.0  # host ignores
                continue
            s0, s1 = int(offs[seg]), int(offs[seg + 1])
            n = s1 - s0
            arr = idx[c, sl * TPS : (sl + 1) * TPS]
            arr[:] = s0
            full, rem = divmod(n, TILE)
            if n:
                arr.reshape(-1)[:n] = np.arange(s0, s1, dtype=np.int64)
            if rem:
                arr[full, rem:] = s0 + full * TILE
            pd = npad[c, sl * TPS : (sl + 1) * TPS]
            pd[:full] = 0.0
            if rem:
                pd[full] = TILE - rem
            pd[full + (1 if rem else 0) :] = TILE
    return dict(
        TPS=TPS,
        SPC=SPC,
        nt=nt,
        idx_flat=idx.reshape(-1),
        npad_bc=np.ascontiguousarray(
            np.broadcast_to(npad[:, None, :], (N_CORES, PB, nt))
        ).reshape(N_CORES * PB, nt),
        counts=counts,
    )


def kernel(
    positions, W1, b1, W2, b2, W3, b3, g1, be1, g2, be2, batch_index, num_segments
):
    import jax

    positions = np.asarray(positions, np.float32)
    weights = tuple(
        np.asarray(a, np.float32) for a in (W1, b1, W2, b2, W3, b3, g1, be1, g2, be2)
    )
    W1, b1, W2, b2, W3, b3, g1, be1, g2, be2 = weights
    bi = np.asarray(batch_index)
    B = int(num_segments)
    st = _STATE

    # ---- packing plan (cached on batch_index equality) ----
    if (
        st.get("bi") is None
        or st.get("B") != B
        or bi.shape != st["bi"].shape
        or not np.array_equal(bi, st["bi"])
    ):
        st["bi"] = bi.copy()
        st["B"] = B
        st["prep"] = _host_prep(st["bi"], B)
        st.pop("pos_copy", None)  # packed positions depend on the plan
        st.pop("w_sig", None)  # npadb rides with the weight upload set
    prep = st["prep"]
    TPS, SPC, nt = prep["TPS"], prep["SPC"], prep["nt"]
    pkey = (TPS, SPC)

    if pkey not in _PROGRAMS:
        _PROGRAMS[pkey] = _build_program(TPS, SPC)
        _RUNNERS[pkey] = _make_runner(_PROGRAMS[pkey])
    run = _RUNNERS[pkey]

    # ---- weights (+ npadb): device-resident, revalidated by equality ----
    w_sig = st.get("w_sig")
    if (
        w_sig is None
        or w_sig[0] != pkey
        or not all(np.array_equal(a, c) for a, c in zip(weights, w_sig[1]))
    ):
        st["w_sig"] = (pkey, tuple(a.copy() for a in weights))
        reps = {
            "w1t": np.ascontiguousarray(np.concatenate([W1.T, b1[None, :]], axis=0)),
            "w2t": np.ascontiguousarray(W2.T),
            "w3t": np.ascontiguousarray(W3.T),
            "b2r": np.ascontiguousarray(b2[None, :]),
            "onesr": np.ones((1, PB), np.float32),
            "gbe": np.ascontiguousarray(np.stack([g1, be1, g2, be2], axis=1)),
        }
        st["w_dev"] = {
            k: jax.device_put(np.concatenate([v] * N_CORES, axis=0), run["shard"])
            for k, v in reps.items()
        }
        st["w_dev"]["npadb"] = jax.device_put(prep["npad_bc"], run["shard"])

    # ---- packed positions: device-resident, revalidated by equality ----
    if (
        st.get("pos_copy") is None
        or positions.shape != st["pos_copy"].shape
        or not np.array_equal(positions, st["pos_copy"])
    ):
        st["pos_copy"] = positions.copy()
        L = nt * TILE
        posT = np.empty((N_CORES, DINA, L), np.float32)
        posT[:, DIN, :] = 1.0
        g = positions[prep["idx_flat"]]
        posT[:, :DIN, :] = g.reshape(N_CORES, L, DIN).transpose(0, 2, 1)
        st["posT_dev"] = jax.device_put(posT.reshape(N_CORES * DINA, L), run["shard"])

    # ---- execute ----
    ins = {"posT": st["posT_dev"], **st["w_dev"]}
    args = [ins[n] for n in run["in_names"]]
    zeros = run["zeros_fn"]()
    outs = run["sharded"](*args, *zeros)
    R = np.asarray(outs[0]).reshape(N_CORES, 6, PB, SPC)

    # ---- host: arrange [core, slot] -> segments, divide, bias ----
    nseg = N_CORES * SPC
    sums = R[:, 0:2].reshape(N_CORES, H, SPC).transpose(0, 2, 1).reshape(nseg, H)[:B]
    maxs = R[:, 2:4].reshape(N_CORES, H, SPC).transpose(0, 2, 1).reshape(nseg, H)[:B]
    mins = R[:, 4:6].reshape(N_CORES, H, SPC).transpose(0, 2, 1).reshape(nseg, H)[:B]
    cnt = np.maximum(prep["counts"], 1).astype(np.float64)
    out = np.empty((B, 3 * H), np.float32)
    out[:, 0:H] = (sums.astype(np.float64) / cnt[:, None]).astype(np.float32) + b3
    out[:, H : 2 * H] = maxs + b3
    out[:, 2 * H : 3 * H] = mins + b3
    return out
